# revision 5
# baseline (speedup 1.0000x reference)
"""Cross-attention kernel for 8 TRN2 NeuronCores (Bass/Tile, SPMD).

Sharding: 8 cores = 2 batches x 4 query-slices (1024 queries each).
Each core computes all 8 heads for its query slice:
  - Q projection for its slice, K/V projections for its batch (replicated
    across the 4 cores of that batch -- on-chip collectives are far too slow
    for the attn2 head-mean reduction, so everything stays core-local).
  - scores are computed transposed [keys, queries]; softmax skips the
    max-subtraction (logits are provably small; a host-side spectral bound
    adds a constant shift if ever needed).  Key-length masking and padding
    are folded into the per-partition bias operand of the Exp activation.
  - A ones-column appended to V yields the softmax denominator from the
    same PSUM accumulation as attn @ V.
  - attn2 (head-mean of attention) accumulates locally: per-head reciprocal
    broadcast via PE outer-product, DVE multiply, GpSimd add.

All matmuls are bf16 with f32 PSUM accumulation; softmax is f32.
"""

import functools
from contextlib import ExitStack

import numpy as np
import ml_dtypes

import concourse.bass as bass
import concourse.mybir as mybir
import concourse.tile as tile
from concourse import bacc
from concourse.bass_utils import run_bass_kernel_spmd
from concourse.masks import make_identity

BF16 = ml_dtypes.bfloat16
P = 128          # SBUF partitions
HEADS = 8
DH = 128         # head dim
DIM = 1024
N_CORES = 8
B = 2
N_Q = 4096       # total queries per batch
Q = N_Q // 4     # queries per core (4 cores per batch)
MASK_NEG = -50000.0


def _emit(ctx, nc, tc, aps, Lp, with_bias):
    """Emit the per-core program. aps: dict of DRAM APs."""
    KC = Lp // P
    f32 = mybir.dt.float32
    bf16 = mybir.dt.bfloat16
    Add = mybir.AluOpType.add
    Mult = mybir.AluOpType.mult

    xqT, xkT, xvT = aps["xqT"], aps["xkT"], aps["xvT"]
    wqT, wkT, wvT = aps["wqT"], aps["wkT"], aps["wvT"]
    mask_bias = aps["mask_bias"]
    out_d, attn2T_d = aps["out"], aps["attn2T"]

    # ---- pools -------------------------------------------------------------
    singles = ctx.enter_context(tc.tile_pool(name="singles", bufs=1))
    wstage = ctx.enter_context(tc.tile_pool(name="wstage", bufs=9))
    xstage = ctx.enter_context(tc.tile_pool(name="xstage", bufs=12))
    persist = ctx.enter_context(tc.tile_pool(name="persist", bufs=1))
    ptpool = ctx.enter_context(tc.tile_pool(name="ptpool", bufs=16))
    small = ctx.enter_context(tc.tile_pool(name="small", bufs=2))
    outst = ctx.enter_context(tc.tile_pool(name="outst", bufs=4))
    tmp2p = ctx.enter_context(tc.tile_pool(name="tmp2p", bufs=3))
    ps_big = ctx.enter_context(tc.tile_pool(name="ps_big", bufs=2, space="PSUM"))
    ps_sm = ctx.enter_context(tc.tile_pool(name="ps_sm", bufs=2, space="PSUM"))

    # ---- constants ---------------------------------------------------------
    mb_sb = singles.tile([P, KC], f32, tag="mb")
    nc.sync.dma_start(out=mb_sb, in_=mask_bias)
    ones8 = singles.tile([1, P], f32, tag="ones8")
    nc.vector.memset(ones8, 1.0 / HEADS)
    ident = singles.tile([P, P], f32, tag="ident")
    make_identity(nc, ident)
    if with_bias:
        bq_sb = singles.tile([P, HEADS], f32, tag="bq")
        nc.sync.dma_start(out=bq_sb, in_=aps["bqT"])
        bk_sb = singles.tile([P, HEADS], f32, tag="bk")
        nc.sync.dma_start(out=bk_sb, in_=aps["bkT"])
        # bv broadcast across partitions: [128, 8, 128]
        bv_sb = singles.tile([P, HEADS, DH], f32, tag="bv")
        bv_src = aps["bv"]
        bv_bcast = bass.AP(
            tensor=bv_src.tensor,
            offset=bv_src.offset,
            ap=[[0, P]] + list(bv_src.ap),
        )
        nc.sync.dma_start(out=bv_sb, in_=bv_bcast)

    # persistent activation tiles
    qT_sb = persist.tile([P, HEADS, Q], bf16, tag="qT")       # q^T, dh on part
    kT_sb = persist.tile([P, HEADS, Lp], bf16, tag="kT")      # k^T, dh on part
    v_ext = persist.tile([P, KC, HEADS, DH + 1], bf16, tag="vx")  # v, keys on part
    acc = persist.tile([P, KC, Q], bf16, tag="acc")           # attn2^T accumulator

    def proj_copy(dst_ap, src_ap, bias_ap):
        if with_bias and bias_ap is not None:
            nc.vector.tensor_scalar_add(dst_ap, src_ap, bias_ap)
        else:
            nc.vector.tensor_copy(dst_ap, src_ap)

    # ---- Phase B: Q projection --------------------------------------------
    # q^T[dh, q] per head:  psum[dh, q] = sum_dc wqT[dc,:,h*DH:+DH].T @ xqT[dc,:,qchunk]
    QCH = Q // 512  # 2
    for qc in range(QCH):
        xq_t = []
        for dc in range(8):
            t = xstage.tile([P, 512], bf16, tag="xs", bufs=12)
            nc.sync.dma_start(out=t, in_=xqT[dc, :, qc * 512:(qc + 1) * 512])
            xq_t.append(t)
        wq_t = []
        for dc in range(8):
            w = wstage.tile([P, DIM], bf16, tag="w", bufs=9)
            nc.sync.dma_start(out=w, in_=wqT[dc])
            wq_t.append(w)
        for hp in range(4):
            ps = ps_big.tile([P, 1024], f32, tag="big", bufs=2)
            for dc in range(8):
                for j in range(2):
                    h = 2 * hp + j
                    nc.tensor.matmul(
                        ps[:, j * 512:(j + 1) * 512],
                        lhsT=wq_t[dc][:, h * DH:(h + 1) * DH],
                        rhs=xq_t[dc],
                        start=(dc == 0),
                        stop=(dc == 7),
                    )
            for j in range(2):
                h = 2 * hp + j
                proj_copy(
                    qT_sb[:, h, qc * 512:(qc + 1) * 512],
                    ps[:, j * 512:(j + 1) * 512],
                    bq_sb[:, h:h + 1] if with_bias else None,
                )

    # ---- Phase C: K projection --------------------------------------------
    kchunks = []
    off = 0
    while off < Lp:
        w = min(512, Lp - off)
        kchunks.append((off, w))
        off += w
    for (koff, kw) in kchunks:
        xk_t = []
        for dc in range(8):
            t = xstage.tile([P, 512], bf16, tag="xs", bufs=12)
            nc.sync.dma_start(out=t[:, :kw], in_=xkT[dc, :, koff:koff + kw])
            xk_t.append(t)
        wk_t = []
        for dc in range(8):
            w_ = wstage.tile([P, DIM], bf16, tag="w", bufs=9)
            nc.sync.dma_start(out=w_, in_=wkT[dc])
            wk_t.append(w_)
        for hp in range(4):
            ps = ps_big.tile([P, 1024], f32, tag="big", bufs=2)
            for dc in range(8):
                for j in range(2):
                    h = 2 * hp + j
                    nc.tensor.matmul(
                        ps[:, j * 512:j * 512 + kw],
                        lhsT=wk_t[dc][:, h * DH:(h + 1) * DH],
                        rhs=xk_t[dc][:, :kw],
                        start=(dc == 0),
                        stop=(dc == 7),
                    )
            for j in range(2):
                h = 2 * hp + j
                proj_copy(
                    kT_sb[:, h, koff:koff + kw],
                    ps[:, j * 512:j * 512 + kw],
                    bk_sb[:, h:h + 1] if with_bias else None,
                )

    # ---- Phase D: V projection --------------------------------------------
    # v[k, (h,dh)]: psum[k-part, 1024] = sum_dc xvT[dc,:,kc*128:+128].T @ wvT[dc]
    VBLK = 3
    for blk in range(0, KC, VBLK):
        kcs = list(range(blk, min(blk + VBLK, KC)))
        nb = len(kcs)
        xv_t = []
        for dc in range(8):
            t = xstage.tile([P, 512], bf16, tag="xs", bufs=12)
            nc.sync.dma_start(
                out=t[:, :nb * P],
                in_=xvT[dc, :, blk * P:(blk + nb) * P],
            )
            xv_t.append(t)
        wv_t = []
        for dc in range(8):
            w_ = wstage.tile([P, DIM], bf16, tag="w", bufs=9)
            nc.sync.dma_start(out=w_, in_=wvT[dc])
            wv_t.append(w_)
        for kc in kcs:
            ps = ps_big.tile([P, 1024], f32, tag="big", bufs=2)
            for dc in range(8):
                for j in range(2):
                    nc.tensor.matmul(
                        ps[:, j * 512:(j + 1) * 512],
                        lhsT=xv_t[dc][:, (kc - blk) * P:(kc - blk + 1) * P],
                        rhs=wv_t[dc][:, j * 512:(j + 1) * 512],
                        start=(dc == 0),
                        stop=(dc == 7),
                    )
            src = ps.rearrange("p (h d) -> p h d", h=HEADS)
            dst = v_ext[:, kc, :, 0:DH]
            if with_bias:
                nc.vector.tensor_tensor(dst, src, bv_sb, op=Add)
            else:
                nc.vector.tensor_copy(dst, src)
    # ones column for the denominator
    nc.vector.memset(v_ext[:, :, :, DH:DH + 1], 1.0)

    # ---- Phase E: attention, per head --------------------------------------
    for h in range(HEADS):
        # scores^T [keys, q] + exp -> pT tiles
        pt_tiles = []
        for kc in range(KC):
            ps = ps_big.tile([P, 1024], f32, tag="big", bufs=2)
            for qc in range(QCH):
                nc.tensor.matmul(
                    ps[:, qc * 512:(qc + 1) * 512],
                    lhsT=kT_sb[:, h, kc * P:(kc + 1) * P],
                    rhs=qT_sb[:, h, qc * 512:(qc + 1) * 512],
                    start=True,
                    stop=True,
                )
            pt = ptpool.tile([P, Q], bf16, tag="pt", bufs=16)
            nc.scalar.activation(
                pt, ps, mybir.ActivationFunctionType.Exp,
                bias=mb_sb[:, kc:kc + 1], scale=1.0,
            )
            pt_tiles.append(pt)

        # attn @ [v | 1] accumulated over key chunks
        recip_all = small.tile([P, HEADS], f32, tag="recip", bufs=2)
        for qs in range(Q // P):  # 8
            po = ps_sm.tile([P, DH + 1], f32, tag="po", bufs=2)
            for kc in range(KC):
                nc.tensor.matmul(
                    po,
                    lhsT=pt_tiles[kc][:, qs * P:(qs + 1) * P],
                    rhs=v_ext[:, kc, h, :],
                    start=(kc == 0),
                    stop=(kc == KC - 1),
                )
            nc.vector.reciprocal(recip_all[:, qs:qs + 1], po[:, DH:DH + 1])
            o_sb = outst.tile([P, DH], f32, tag="osb", bufs=4)
            nc.scalar.activation(
                o_sb, po[:, 0:DH], mybir.ActivationFunctionType.Copy,
                bias=0.0, scale=recip_all[:, qs:qs + 1],
            )
            nc.sync.dma_start(
                out=out_d[qs * P:(qs + 1) * P, h * DH:(h + 1) * DH],
                in_=o_sb,
            )

        # broadcast (1/8)/denom along partitions: transpose + outer product
        row_sb = small.tile([1, Q], f32, tag="row_sb", bufs=2)
        for half in range(Q // 512):
            row_ps = ps_sm.tile([1, 512], f32, tag="row", bufs=1)
            for j in range(4):
                qs = half * 4 + j
                nc.tensor.transpose(
                    row_ps[0:1, j * P:(j + 1) * P],
                    recip_all[:, qs:qs + 1],
                    ident,
                )
            nc.scalar.copy(row_sb[0:1, half * 512:(half + 1) * 512], row_ps)
        bcast = small.tile([P, Q], bf16, tag="bcast", bufs=2)
        for half in range(Q // 512):
            pb = ps_sm.tile([P, 512], f32, tag="pb", bufs=1)
            nc.tensor.matmul(
                pb,
                lhsT=ones8,
                rhs=row_sb[0:1, half * 512:(half + 1) * 512],
                start=True,
                stop=True,
            )
            nc.vector.tensor_copy(bcast[:, half * 512:(half + 1) * 512], pb)

        # attn2^T accumulation
        for kc in range(KC):
            if h == 0:
                nc.vector.tensor_tensor(acc[:, kc, :], pt_tiles[kc], bcast, op=Mult)
            else:
                t2 = tmp2p.tile([P, Q], bf16, tag="t2", bufs=3)
                nc.vector.tensor_tensor(t2, pt_tiles[kc], bcast, op=Mult)
                nc.gpsimd.tensor_tensor(acc[:, kc, :], acc[:, kc, :], t2, op=Add)

    # ---- Phase F: attn2 out ------------------------------------------------
    for kc in range(KC):
        nc.sync.dma_start(out=attn2T_d[kc], in_=acc[:, kc, :])


@functools.lru_cache(maxsize=4)
def build(Lp, with_bias):
    """Build + compile the SPMD program (identical on all 8 cores)."""
    KC = Lp // P
    nc = bacc.Bacc(
        "TRN2", target_bir_lowering=False, debug=False, num_devices=N_CORES
    )
    dt = mybir.dt
    aps = {}
    aps["xqT"] = nc.dram_tensor("xqT", [8, P, Q], dt.bfloat16, kind="ExternalInput").ap()
    aps["xkT"] = nc.dram_tensor("xkT", [8, P, Lp], dt.bfloat16, kind="ExternalInput").ap()
    aps["xvT"] = nc.dram_tensor("xvT", [8, P, Lp], dt.bfloat16, kind="ExternalInput").ap()
    aps["wqT"] = nc.dram_tensor("wqT", [8, P, DIM], dt.bfloat16, kind="ExternalInput").ap()
    aps["wkT"] = nc.dram_tensor("wkT", [8, P, DIM], dt.bfloat16, kind="ExternalInput").ap()
    aps["wvT"] = nc.dram_tensor("wvT", [8, P, DIM], dt.bfloat16, kind="ExternalInput").ap()
    aps["mask_bias"] = nc.dram_tensor("mask_bias", [P, KC], dt.float32, kind="ExternalInput").ap()
    if with_bias:
        aps["bqT"] = nc.dram_tensor("bqT", [P, HEADS], dt.float32, kind="ExternalInput").ap()
        aps["bkT"] = nc.dram_tensor("bkT", [P, HEADS], dt.float32, kind="ExternalInput").ap()
        aps["bv"] = nc.dram_tensor("bv", [HEADS, DH], dt.float32, kind="ExternalInput").ap()
    aps["out"] = nc.dram_tensor("out", [Q, DIM], dt.float32, kind="ExternalOutput").ap()
    aps["attn2T"] = nc.dram_tensor("attn2T", [KC, P, Q], dt.bfloat16, kind="ExternalOutput").ap()

    with tile.TileContext(nc) as tc:
        with ExitStack() as ctx:
            _emit(ctx, nc, tc, aps, Lp, with_bias)
    nc.compile()
    return nc


def _logit_bound(Wq_s, Wk, max_xq, max_xk):
    """Upper bound on |scores| via per-head spectral norms (power iteration).
    Wq_s already includes the 1/sqrt(dim) scale."""
    rng = np.random.RandomState(0)
    bound = 0.0
    for h in range(HEADS):
        A = Wq_s[h * DH:(h + 1) * DH].astype(np.float64)   # [128, 1024]
        Bm = Wk[h * DH:(h + 1) * DH].astype(np.float64)
        sig = []
        for M in (A, Bm):
            v = rng.randn(M.shape[1])
            v /= np.linalg.norm(v)
            s = 0.0
            for _ in range(20):
                u = M @ v
                s = np.linalg.norm(u)
                if s == 0:
                    break
                u /= s
                v = M.T @ u
                nv = np.linalg.norm(v)
                if nv == 0:
                    break
                v /= nv
            sig.append(s * 1.25)  # convergence margin
        bound = max(bound, sig[0] * sig[1] * max_xq * max_xk)
    return bound


def _prep_in_maps(inputs, Lp):
    """Host-side sharding/layout. inputs: dict like setup_inputs()."""
    queries = np.asarray(inputs["queries"], dtype=np.float32)
    keys = np.asarray(inputs["keys"], dtype=np.float32)
    values = np.asarray(inputs["values"], dtype=np.float32)
    Wq = np.asarray(inputs["Wq"], dtype=np.float32)
    Wk = np.asarray(inputs["Wk"], dtype=np.float32)
    Wv = np.asarray(inputs["Wv"], dtype=np.float32)
    bq = np.asarray(inputs["bq"], dtype=np.float32)
    bk = np.asarray(inputs["bk"], dtype=np.float32)
    bv = np.asarray(inputs["bv"], dtype=np.float32)
    n2 = keys.shape[1]
    lens = np.clip(np.asarray(inputs["len_x"]).astype(np.int64), 1, n2)
    KC = Lp // P
    dim = DIM

    with_bias = bool(np.any(bq) or np.any(bk) or np.any(bv))
    scale = 1.0 / np.sqrt(DIM)

    # numeric safety: constant shift if logits could overflow exp
    Wq_s = Wq * scale
    max_xq = float(np.max(np.linalg.norm(queries.reshape(-1, dim), axis=1)))
    max_xk = float(np.max(np.linalg.norm(keys[:, :Lp].reshape(-1, dim), axis=1)))
    bound = _logit_bound(Wq_s, Wk, max_xq, max_xk)
    if with_bias:
        bound += float(np.max(np.abs(bq))) * scale * (
            float(np.linalg.norm(bk)) + 2.0 * max_xk
        ) + float(np.linalg.norm(bk)) * max_xq * scale * 2.0
    C = max(0.0, bound - 80.0)

    wqT_h = np.ascontiguousarray(Wq_s.T).astype(BF16).reshape(8, P, DIM)
    wkT_h = np.ascontiguousarray(Wk.T).astype(BF16).reshape(8, P, DIM)
    wvT_h = np.ascontiguousarray(Wv.T).astype(BF16).reshape(8, P, DIM)

    xkT_b, xvT_b, mask_b = [], [], []
    for bi in range(B):
        kpad = np.zeros((Lp, DIM), np.float32)
        vpad = np.zeros((Lp, DIM), np.float32)
        Luse = min(int(lens[bi]), Lp)
        kpad[:Luse] = keys[bi, :Luse]
        vpad[:Luse] = values[bi, :Luse]
        xkT_b.append(np.ascontiguousarray(kpad.T).astype(BF16).reshape(8, P, Lp))
        xvT_b.append(np.ascontiguousarray(vpad.T).astype(BF16).reshape(8, P, Lp))
        mb = np.full((KC * P,), MASK_NEG, np.float32)
        mb[:Luse] = 0.0
        mb -= C
        mask_b.append(np.ascontiguousarray(mb.reshape(KC, P).T))  # [128, KC]

    in_maps = []
    for core in range(N_CORES):
        bi, sl = core // 4, core % 4
        xq = queries[bi, sl * Q:(sl + 1) * Q, :]          # [Q, DIM]
        xqT = np.ascontiguousarray(xq.T).astype(BF16).reshape(8, P, Q)
        m = {
            "xqT": xqT,
            "xkT": xkT_b[bi],
            "xvT": xvT_b[bi],
            "wqT": wqT_h,
            "wkT": wkT_h,
            "wvT": wvT_h,
            "mask_bias": mask_b[bi],
        }
        if with_bias:
            m["bqT"] = np.ascontiguousarray((bq * scale).reshape(8, P).T)
            m["bkT"] = np.ascontiguousarray(bk.reshape(8, P).T)
            m["bv"] = np.ascontiguousarray(bv.reshape(HEADS, DH))
        in_maps.append(m)
    return in_maps, with_bias, lens


def kernel(queries, keys, values, Wq, bq, Wk, bk, Wv, bv, len_x, lgt):
    queries = np.asarray(queries, dtype=np.float32)
    keys = np.asarray(keys, dtype=np.float32)
    b, n, dim = queries.shape
    _, n2, _ = keys.shape
    assert (b, n, dim, n2) == (B, N_Q, DIM, 2048), "kernel hardcoded for this shape"

    lens = np.clip(np.asarray(len_x).astype(np.int64), 1, n2)
    Lmax = int(lens.max())
    Lp = ((Lmax + P - 1) // P) * P

    inputs = dict(queries=queries, keys=keys, values=values, Wq=Wq, bq=bq,
                  Wk=Wk, bk=bk, Wv=Wv, bv=bv, len_x=len_x, lgt=lgt)
    in_maps, with_bias, lens = _prep_in_maps(inputs, Lp)

    nc = build(Lp, with_bias)
    res = run_bass_kernel_spmd(nc, in_maps, core_ids=list(range(N_CORES)))

    out = np.empty((B, N_Q, DIM), np.float32)
    attn2 = np.zeros((B, N_Q, n2), np.float32)
    for core in range(N_CORES):
        bi, sl = core // 4, core % 4
        r = res.results[core]
        out[bi, sl * Q:(sl + 1) * Q, :] = r["out"]
        a2 = np.asarray(r["attn2T"]).reshape(Lp, Q).astype(np.float32)
        attn2[bi, sl * Q:(sl + 1) * Q, :Lp] = a2.T
        attn2[bi, :, int(lens[bi]):] = 0.0
    return out, attn2
